# revision 4
# baseline (speedup 1.0000x reference)
"""Bidirectional Mamba (DSS) selective-scan kernel for 8 Trainium2 cores.

Sharding: core = (batch b, direction dir). 4 batches x 2 directions = 8
independent shards; zero collectives. Backward-direction cores consume the
time-reversed sequence (host prep) and their partial output is un-reversed
on the host. Each core computes its direction's half of the output
projection (y_dir @ Wo_half.T); host sums the two partials per batch.

Device layout: [d on partitions, t on free]. The selective scan runs on the
DVE hardware scan instruction (TensorTensorScanArith):
    state = a_t * state + b_t      along the free (time) axis
with a_t = exp(A[d,n] * delta_t[d]) built on the Scalar engine
(delta = softplus(raw) computed as -ln(sigmoid(-raw))), and
b_t = delta_t*u_t*B_t[n] built on the Vector engine with B broadcast
across partitions via stride-0 DMA from a DRAM scratch row.
"""

import numpy as np

import concourse.bass as bass
import concourse.mybir as mybir
import concourse.tile as tile

F32 = mybir.dt.float32
F16 = mybir.dt.float16
AF = mybir.ActivationFunctionType
OP = mybir.AluOpType

# problem constants (hardcoded per contract)
B, L, DM = 4, 4096, 256
DI = 512          # d_inner
NS = 16           # d_state
DT_RANK = 16
NCORES = 8
NDD = DI // 128   # 4 d-inner tiles of 128
NMT = 2 * DM // 256  # output row tiles (256 rows = 2 x 128)

TC = 512          # scan time-chunk
NTC = L // TC

_cache = {}


def _build_bass():
    nc = bass.Bass()
    hT = nc.dram_tensor("hT", [2 * 128, L], F16, kind="ExternalInput")
    WiT = nc.dram_tensor("WiT", [2 * 128, 2 * DI], F16, kind="ExternalInput")
    WxT = nc.dram_tensor("WxT", [DI, DT_RANK + 2 * NS], F16, kind="ExternalInput")
    WdtA = nc.dram_tensor("WdtA", [DT_RANK, DI], F16, kind="ExternalInput")
    WoT = nc.dram_tensor("WoT", [DI, 2 * 128], F16, kind="ExternalInput")
    cols = nc.dram_tensor("cols", [128, 96], F32, kind="ExternalInput")
    outT = nc.dram_tensor("outT", [2 * 128, L], F32, kind="ExternalOutput")

    with tile.TileContext(nc) as tc:
        _emit(nc, tc, hT, WiT, WxT, WdtA, WoT, cols, outT)

    import sys, os
    sys.path.insert(0, os.path.dirname(os.path.abspath(__file__)))
    from bass_fix import split_excess_waits

    split_excess_waits(nc)
    return nc


def _emit(nc, tc, hT, WiT, WxT, WdtA, WoT, cols, outT):
    with (
        tc.tile_pool(name="pers", bufs=1) as pers,
        tc.tile_pool(name="wpool", bufs=1) as wpool,
        tc.tile_pool(name="bcast", bufs=2) as bcast,
        tc.tile_pool(name="tmp", bufs=3) as tmp,
        tc.tile_pool(name="scan", bufs=2) as scan_pool,
        tc.tile_pool(name="ostg", bufs=2) as ostg,
        tc.tile_pool(name="psum", bufs=4, space="PSUM") as psum,
        tc.tile_pool(name="dram", bufs=1, space="DRAM") as dram,
    ):
        # ---- load weights / constants ----
        wi = wpool.tile([128, 2, 2 * DI], F16, name="wi")     # Wi^T 2 K-chunks
        nc.sync.dma_start(out=wi[:], in_=WiT[:].rearrange("(k p) m -> p k m", p=128))
        wx = wpool.tile([128, NDD, DT_RANK + 2 * NS], F16, name="wx")
        nc.sync.dma_start(out=wx[:], in_=WxT[:].rearrange("(k p) m -> p k m", p=128))
        wdt = wpool.tile([DT_RANK, DI], F16, name="wdt")
        nc.sync.dma_start(out=wdt[:], in_=WdtA[:])
        wo = wpool.tile([128, NDD, 2 * 128], F16, name="wo")
        nc.sync.dma_start(out=wo[:], in_=WoT[:].rearrange("(k p) m -> p k m", p=128))
        colt = wpool.tile([128, 96], F32, name="colt")
        nc.sync.dma_start(out=colt[:], in_=cols[:])

        ht = [bcast.tile([128, L], F16, name=f"ht{k}", tag=["bbc","cbc"][k]) for k in range(2)]
        for k in range(2):
            nc.sync.dma_start(out=ht[k][:], in_=hT[128 * k : 128 * (k + 1), :])

        # ---- P1: xz^T = Wi @ h  ->  u = silu(x), sz = silu(z) ----
        u = [pers.tile([128, L], F16, name=f"u{dd}") for dd in range(NDD)]
        sz = [pers.tile([128, L], F16, name=f"sz{dd}") for dd in range(NDD)]
        for j in range(2 * NDD):  # 8 output row-tiles of xz^T
            dst = u[j] if j < NDD else sz[j - NDD]
            for c in range(L // 512):
                pt = psum.tile([128, 512], F32, name="p1", tag="ps")
                for k in range(2):
                    nc.tensor.matmul(
                        pt[:],
                        wi[:, k, 128 * j : 128 * (j + 1)],
                        ht[k][:, 512 * c : 512 * (c + 1)],
                        start=(k == 0),
                        stop=(k == 1),
                    )
                nc.scalar.activation(
                    dst[:, 512 * c : 512 * (c + 1)], pt[:], AF.Silu
                )

        # ---- P2: x_dbl^T = Wx @ u  -> rows 0:16 dtraw, 16:32 B, 32:48 C ----
        xdbl = pers.tile([48, L], F16, name="xdbl")
        for c in range(L // 512):
            pt = psum.tile([48, 512], F32, name="p2", tag="ps")
            for k in range(NDD):
                nc.tensor.matmul(
                    pt[:],
                    wx[:, k, :],
                    u[k][:, 512 * c : 512 * (c + 1)],
                    start=(k == 0),
                    stop=(k == NDD - 1),
                )
            nc.scalar.copy(xdbl[:, 512 * c : 512 * (c + 1)], pt[:, :])

        # stash B/C rows to DRAM for stride-0 partition broadcast
        bc_rows = dram.tile([32, L], F16, name="bc_rows")
        nc.sync.dma_start(out=bc_rows[:], in_=xdbl[16:48, :])

        # ---- P3: delta path ----
        # negdelta = ln(sigmoid(-(Wdt@dtraw + bdt))) = -softplus(raw)
        negd = [pers.tile([128, L], F16, name=f"negd{dd}") for dd in range(NDD)]
        w = [pers.tile([128, L], F16, name=f"w{dd}") for dd in range(NDD)]
        for dd in range(NDD):
            for c in range(L // 512):
                pt = psum.tile([128, 512], F32, name="p3", tag="ps")
                nc.tensor.matmul(
                    pt[:],
                    wdt[:, 128 * dd : 128 * (dd + 1)],
                    xdbl[0:16, 512 * c : 512 * (c + 1)],
                    start=True,
                    stop=True,
                )
                sg = tmp.tile([128, 512], F32, name="sg")
                nc.scalar.activation(
                    sg[:], pt[:], AF.Sigmoid, scale=-1.0,
                    bias=colt[:, 68 + dd : 69 + dd],
                )
                nc.scalar.activation(
                    negd[dd][:, 512 * c : 512 * (c + 1)], sg[:], AF.Ln
                )
            # w = delta * u = (-negd) * u
            for c in range(NTC):
                sl = slice(TC * c, TC * (c + 1))
                nc.vector.scalar_tensor_tensor(
                    out=w[dd][:, sl],
                    in0=negd[dd][:, sl],
                    scalar=-1.0,
                    in1=u[dd][:, sl],
                    op0=OP.mult,
                    op1=OP.mult,
                )
            # y_acc := D * u  (in place on u; u dead after this)
            nc.vector.tensor_scalar_mul(u[dd][:], u[dd][:], colt[:, 64 + dd : 65 + dd])
        y_acc = u  # renamed: accumulates sum_n C_n * s_n + D*u

        # ---- P4: the scan ----
        for n in range(NS):
            rb = bc_rows[n : n + 1, :]
            bb = bcast.tile([128, L], F16, name="bbc", tag="bbc")
            nc.sync.dma_start(
                out=bb[:],
                in_=bass.AP(tensor=rb.tensor, offset=rb.offset, ap=[[0, 128]] + list(rb.ap[1:])),
            )
            rc = bc_rows[16 + n : 17 + n, :]
            cb = bcast.tile([128, L], F16, name="cbc", tag="cbc")
            nc.sync.dma_start(
                out=cb[:],
                in_=bass.AP(tensor=rc.tensor, offset=rc.offset, ap=[[0, 128]] + list(rc.ap[1:])),
            )
            for dd in range(NDD):
                s_prev = None
                for c in range(NTC):
                    sl = slice(TC * c, TC * (c + 1))
                    a_t = scan_pool.tile([128, TC], F32, name="a_t", tag="a_t")
                    nc.scalar.activation(
                        a_t[:], negd[dd][:, sl], AF.Exp,
                        scale=colt[:, 16 * dd + n : 16 * dd + n + 1],
                    )
                    b_t = scan_pool.tile([128, TC], F32, name="b_t", tag="b_t")
                    nc.vector.tensor_mul(b_t[:], w[dd][:, sl], bb[:, sl])
                    s_t = scan_pool.tile([128, TC], F16, name="s_t", tag="s_t")
                    nc.vector.tensor_tensor_scan(
                        s_t[:], a_t[:], b_t[:],
                        0.0 if s_prev is None else s_prev[:, TC - 1 : TC],
                        OP.mult, OP.add,
                    )
                    s_prev = s_t
                    yt = scan_pool.tile([128, TC], F16, name="yt", tag="yt")
                    nc.vector.tensor_mul(yt[:], s_t[:], cb[:, sl])
                    nc.vector.tensor_add(y_acc[dd][:, sl], y_acc[dd][:, sl], yt[:])

        # ---- P5: gate ----
        for dd in range(NDD):
            nc.vector.tensor_mul(y_acc[dd][:], y_acc[dd][:], sz[dd][:])

        # ---- P6: out_partial^T = Wo_half @ y ----
        for m in range(2):
            for c in range(L // 512):
                pt = psum.tile([128, 512], F32, name="p6", tag="ps")
                for k in range(NDD):
                    nc.tensor.matmul(
                        pt[:],
                        wo[:, k, 128 * m : 128 * (m + 1)],
                        y_acc[k][:, 512 * c : 512 * (c + 1)],
                        start=(k == 0),
                        stop=(k == NDD - 1),
                    )
                og = ostg.tile([128, 512], F32, name="og")
                nc.scalar.copy(og[:], pt[:])
                nc.sync.dma_start(
                    out=outT[128 * m : 128 * (m + 1), 512 * c : 512 * (c + 1)],
                    in_=og[:],
                )


def _prep_inputs(inputs):
    """Build the 8 per-core input maps from the full inputs."""
    h = np.asarray(inputs["hidden_states"], np.float32)
    maps = []
    for core in range(NCORES):
        b, dir_ = core % 4, core // 4
        tag = "f" if dir_ == 0 else "b"
        Wi = np.asarray(inputs[f"Wi_{tag}"], np.float32)
        Wx = np.asarray(inputs[f"Wx_{tag}"], np.float32)
        Wdt = np.asarray(inputs[f"Wdt_{tag}"], np.float32)
        bdt = np.asarray(inputs[f"bdt_{tag}"], np.float32)
        A_log = np.asarray(inputs[f"A_log_{tag}"], np.float32)
        D = np.asarray(inputs[f"D_{tag}"], np.float32)
        Wo = np.asarray(inputs["Wo"], np.float32)

        hb = h[b] if dir_ == 0 else h[b][::-1]
        cols = np.zeros((128, 96), np.float32)
        Aexp = np.exp(A_log)  # [512, 16] positive; a = exp(A*delta)=exp(Aexp*negd)
        for dd in range(NDD):
            cols[:, 16 * dd : 16 * (dd + 1)] = Aexp[128 * dd : 128 * (dd + 1), :]
            cols[:, 64 + dd] = D[128 * dd : 128 * (dd + 1)]
            cols[:, 68 + dd] = -bdt[128 * dd : 128 * (dd + 1)]
        maps.append(
            {
                "hT": np.ascontiguousarray(hb.T).astype(np.float16),
                "WiT": np.ascontiguousarray(Wi.T).astype(np.float16),
                "WxT": np.ascontiguousarray(Wx.T).astype(np.float16),
                "WdtA": np.ascontiguousarray(Wdt.T).astype(np.float16),
                "WoT": np.ascontiguousarray(
                    Wo[:, 512 * dir_ : 512 * (dir_ + 1)].T
                ).astype(np.float16),
                "cols": cols,
            }
        )
    return maps


def kernel(**inputs):
    from concourse.bass_utils import run_bass_kernel_spmd

    if "nc" not in _cache:
        _cache["nc"] = _build_bass()
    nc = _cache["nc"]
    maps = _prep_inputs(inputs)
    res = run_bass_kernel_spmd(nc, maps, list(range(NCORES)))
    out = np.empty((B, L, DM), np.float32)
    for b in range(B):
        f = res.results[b]["outT"]          # [256, L]
        r = res.results[b + 4]["outT"]      # [256, L] (time-reversed)
        out[b] = (f + r[:, ::-1]).T
    return out


if __name__ == "__main__":
    rng = np.random.default_rng(0)
    demo = {"hidden_states": rng.standard_normal((B, L, DM)).astype(np.float32)}
    print("built bass module ok")
